# revision 33
# baseline (speedup 1.0000x reference)
"""Trainium2 Bass kernel for BudgetAttentionTwo (v5).

Module: keys = x@Wk.T+bk, values = x@Wv.T+bv (split into 8 heads of 64),
S = K K^T per (b, h), out = (softmax(S)/sqrt(E)) @ V, merged back to [B,N,E].

Sharding: 8 cores, each core owns one batch b = core//2 and four heads
hg*4..hg*4+3 (hg = core%2). No cross-device comms.

History (per-core HW time):
  v3 197.9us -> v4 187.8 (fp16 inputs/outputs, contiguous DMA layouts)
  -> v8 165.3 -> v11 161.4. The steady state is ACT-bound: exp of 16.8M
  scores = 96 ACTIVATEs x (FD+296 cyc)/1.2GHz = 132.9us with <1us of
  stalls; the rest is startup (~16us: preamble 6, DMA transfer+receipt
  ~7, first kproj/score chain ~3) and tail (~12: last attV + DVE
  normalize chain + DMA receipt + ~5us teardown).

Key mechanics (learned the hard way, via ntff traces):
  - attV tail chunks (12..15) of iteration k-1 run after iteration k's
    g0/g1 score groups (pts pool bufs=3) so PE detours never starve ACT;
    groups [2,3,3,3,3,2] make the boundary group cheap.
  - ALL projections live in iteration 0 on the "av" psum banks (attV has
    no accumulator yet), so kproj/vproj never displace a score buffer.
  - reciprocal_approx_fast (custom DVE, base partition 0 ONLY -- offset
    64 returns garbage on HW) + bf16 cast; the broadcast matmul must be
    bf16 (fp32 matmuls lower to a HI/LO pair at ~3x cost).
  - K=64 sub-array score matmuls (tile_position row 64) corrupt on HW
    even though CoreSim accepts them -- hence the bd zero-half tensors.
  - 8 warm-up matmuls on garbage bring the PE HAM clock gate to 2.4GHz
    before the first real projection.
  - DMA: fp16, partition-major 4KB lines; first 8 dma_starts get the 8
    completion lanes; ~2.5us receipt latency per transfer.
Numerics: fp16 x/W projections measured 5.04e-3 rel err (tolerance 2e-2),
bit-identical to the numpy simulation of the dtype chain.

P is bf16, V (with trailing ones column) bf16, K fp16. exp(S - 88) is
exact for softmax (max logit ~131); rowsums via the ones column; output
stays transposed [64 d, N] fp16 per head; host transposes and upcasts.
"""
import numpy as np

import concourse.bacc as bacc
import concourse.mybir as mybir
import concourse.tile as tile
from concourse.bass_utils import run_bass_kernel_spmd

F32 = mybir.dt.float32
BF16 = mybir.dt.bfloat16
F16 = mybir.dt.float16
EXP = mybir.ActivationFunctionType.Exp

B, N, E, H = 4, 2048, 512, 8
D = E // H            # 64
NCORES = 8
HPC = 4               # heads per core
CSHIFT = 88.0         # exp(S - CSHIFT)
QW = 512              # q-range width
NS = N // QW          # 4 q-ranges
KC = N // 128         # 16 k-chunks
GRPW = 3              # max k-chunks per psum tile / exp call
GROUPS = [(0, 2), (2, 3), (5, 3), (8, 3), (11, 3), (14, 2)]
# attV chunks of the accumulator begun last iteration, emitted after score
# groups g1..g4; the (12..15) tail runs after the NEXT iteration's g0.
AV_PLAN = [(0, 1, 2), (3, 4, 5), (6, 7, 8), (9, 10, 11)]
AV_TAIL = (12, 13, 14, 15)

_last_results = None  # stashed BassKernelResults for test.py introspection


def _register_const(nc, val):
    """Extra pre-TileContext f32 [128,1] constant (dep-free, like Bass's
    built-in consts) so activation(bias=val) needs no semaphore wait."""
    t = nc.alloc_sbuf_tensor(f"const-float32-{val}", [128, 1], F32)
    nc.gpsimd.memset(t.ap(), val)
    nc.const_aps.aps[(F32, float(val))] = t.ap()
    # no all_engine_barrier: the memset retires ~6us into the gpsimd
    # preamble, ~9us before the first ACTIVATE that reads it; the explicit
    # barrier cost 1.4us of startup


def build_program():
    nc = bacc.Bacc()
    _register_const(nc, -CSHIFT)

    xt4 = nc.dram_tensor("xt4", [NS, 128, 4, QW], F16, kind="ExternalInput")
    wkv = nc.dram_tensor("wkv", [128, 4, 512], F16, kind="ExternalInput")
    bk2 = nc.dram_tensor("bk2", [2, 128, 1], F32, kind="ExternalInput")
    bvb = nc.dram_tensor("bvb", [128, 2 * 128], F32, kind="ExternalInput")
    out_t = nc.dram_tensor("out_t", [HPC, D, N], F16, kind="ExternalOutput")

    with nc.allow_low_precision(reason="fp16/bf16 datapath is intentional"), \
         tile.TileContext(nc) as tc:
        with (
            tc.tile_pool(name="persist", bufs=1) as per,
            tc.tile_pool(name="work", bufs=2) as work,
            tc.tile_pool(name="mps", bufs=1, space="PSUM") as mps,
        ):
            # ---- persistent SBUF ----
            kt2 = [per.tile([128, N], F16, name=f"kt2_{p}") for p in range(2)]
            bd = [[per.tile([128, N], F16, name=f"bd_{j}_{p}")
                   for p in range(2)] for j in range(2)]
            vs = [per.tile([128, HPC * (D + 1)], BF16, name=f"vs_{t}")
                  for t in range(KC)]
            bvb_sb = per.tile([128, HPC * D], F32)
            bk_sb = [per.tile([128, 1], F32, name=f"bk_{p}") for p in range(2)]
            # ones rows at partitions 0 and 32 for the two broadcast
            # matmuls (sub-128 partition bases beyond 32 misbehave on HW);
            # bf16 so the broadcast matmul stays a single cheap instruction
            # (fp32 matmuls lower to a HI/LO pair at ~3x the cost)
            ones33 = per.tile([33, D], BF16)

            nc.gpsimd.memset(ones33[:], 1.0)
            # bd zero halves: only the p=0 pair is needed early (first
            # scores ~12us); p=1 isn't read until iteration 4 (~75us), so
            # those go on gpsimd AFTER its DMA issues (emitted below).
            # Keeping DVE to 2 memsets frees it for the first kproj add.
            nc.vector.memset(bd[0][0][64:128, :], 0.0)
            nc.vector.memset(bd[1][0][0:64, :], 0.0)

            def kproj(p, qr):
                # psum from the "av" tag: free during iteration 0 (no attV
                # yet), so projections never displace a score buffer
                acc = mps.tile([128, QW], F32, tag="av", bufs=2,
                               name=f"kacc_{p}_{qr}")
                for c in range(4):
                    nc.tensor.matmul(
                        acc[:],
                        wkv_sb[c][:, 128 * p:128 * (p + 1)],
                        xt_sb[c][:, QW * qr:QW * (qr + 1)],
                        start=(c == 0), stop=(c == 3),
                    )
                qs = slice(QW * qr, QW * (qr + 1))
                nc.vector.tensor_scalar_add(kt2[p][:, qs], acc[:],
                                            bk_sb[p][:])
                # bd halves: fast DVE f16 copies from kt2 (4x perf mode),
                # off the psum (which the single add above already drained)
                nc.vector.tensor_copy(bd[0][p][0:64, qs], kt2[p][0:64, qs])
                nc.vector.tensor_copy(bd[1][p][64:128, qs],
                                      kt2[p][64:128, qs])

            def vproj(t):
                acc = mps.tile([128, QW], F32, tag="av", bufs=2,
                               name=f"vacc_{t}")
                for c in range(4):
                    nc.tensor.matmul(
                        acc[:, :HPC * D],
                        xt_sb[c][:, 128 * t:128 * (t + 1)],
                        wkv_sb[c][:, 256:512],
                        start=(c == 0), stop=(c == 3),
                    )
                vst = vs[t].rearrange("p (h y) -> p h y", h=HPC)
                nc.gpsimd.memset(vst[:, :, D], 1.0)
                nc.vector.tensor_tensor(
                    out=vst[:, :, 0:D],
                    in0=acc[:, :HPC * D].rearrange("p (h d) -> p h d", h=HPC),
                    in1=bvb_sb.rearrange("p (h d) -> p h d", h=HPC),
                    op=mybir.AluOpType.add,
                )

            def scores_group(p, qr, gw, pts):
                """Score matmuls + exp for k-chunks g..g+w-1 of (p, qr).
                Full K=128 contraction with the bd zero-half trick (K=64
                sub-array matmuls are broken on HW; CoreSim disagrees)."""
                g, w = gw
                sc = [mps.tile([128, GRPW * QW], F32, tag="sc", bufs=2,
                               name=f"sc_{p}_{qr}_{g}_{j}")
                      for j in range(2)]
                for i in range(w):
                    kc = g + i
                    for j in range(2):
                        nc.tensor.matmul(
                            sc[j][:, QW * i:QW * (i + 1)],
                            kt2[p][:, 128 * kc:128 * (kc + 1)],
                            bd[j][p][:, QW * qr:QW * (qr + 1)],
                            start=True, stop=True,
                        )
                for j in range(2):
                    nc.scalar.activation(
                        pts[j][:, QW * g:QW * (g + w)],
                        sc[j][:, :QW * w],
                        EXP, bias=-CSHIFT, scale=1.0,
                    )

            def attv_begin(p, cc, pts):
                return {
                    "p": p, "cc": cc, "pts": pts, "n": [0, 0],
                    "av": [mps.tile([128, QW], F32, tag="av", bufs=2,
                                    name=f"av_{p}_{cc}_{j}")[0:D + 1, :]
                           for j in range(2)],
                }

            def attv_chunks(st, kcs):
                p = st["p"]
                for kc in kcs:
                    vsl = vs[kc].rearrange("p (h y) -> p h y", h=HPC)
                    for j in range(2):
                        nc.tensor.matmul(
                            st["av"][j][:], vsl[:, 2 * p + j, :],
                            st["pts"][j][:, QW * kc:QW * (kc + 1)],
                            start=(st["n"][j] == 0),
                            stop=(st["n"][j] == KC - 1),
                        )
                        st["n"][j] += 1

            def attv_copies(st):
                p, cc = st["p"], st["cc"]
                assert st["n"] == [KC, KC]
                avs = []
                rb = work.tile([33, QW], F32, tag="rb", bufs=2,
                               name=f"rb_{p}_{cc}")
                for j in range(2):
                    av_sb = work.tile([D + 1, QW], F32, tag="avsb", bufs=4,
                                      name=f"avsb_{p}_{cc}_{j}")
                    nc.vector.tensor_copy(av_sb[:], st["av"][j][:])
                    nc.vector.tensor_copy(rb[32 * j:32 * j + 1, :],
                                          av_sb[D:D + 1, :])
                    avs.append(av_sb)
                return (p, cc, avs, rb)

            def attv_recip(stc):
                """Reciprocal of both rowsum rows (lanes 0 and 32; lanes
                1..31 hold junk whose reciprocal is never read), then a
                bf16 cast so the broadcast matmul stays single-instruction."""
                p, cc, avs, rb = stc
                rr = work.tile([33, QW], F32, tag="rr", bufs=2,
                               name=f"rr_{p}_{cc}")
                nc.vector.reciprocal_approx_fast(rr[:], rb[:])
                rrb = work.tile([33, QW], BF16, tag="rrb", bufs=2,
                                name=f"rrb_{p}_{cc}")
                nc.vector.tensor_copy(rrb[:], rr[:])
                return (p, cc, avs, rrb)

            def epilogue(state, last=False):
                p, cc, avs, rr = state
                q0 = QW * cc
                for j in range(2):
                    hl = 2 * p + j
                    if last:
                        # scores are done; the freed "sc" banks host the
                        # broadcast so it needn't wait for the av rotation
                        bc = mps.tile([128, GRPW * QW], F32, tag="sc",
                                      bufs=2,
                                      name=f"bc_{p}_{cc}_{j}")[0:D, 0:QW]
                    else:
                        bc = mps.tile([128, QW], F32, tag="av", bufs=2,
                                      name=f"bc_{p}_{cc}_{j}")[0:D, :]
                    nc.tensor.matmul(bc[:], ones33[32 * j:32 * j + 1, :],
                                     rr[32 * j:32 * j + 1, :],
                                     start=True, stop=True)
                    fin = work.tile([D, QW], F16, tag="fin", bufs=2,
                                    name=f"fin_{p}_{cc}_{j}")
                    nc.vector.tensor_tensor(
                        out=fin[:], in0=avs[j][0:D, :], in1=bc[:],
                        op=mybir.AluOpType.mult)
                    eng = nc.scalar if last else nc.sync
                    eng.dma_start(
                        out=out_t[hl, :, q0:q0 + QW], in_=fin[:])

            ITERS = [(p, qr) for p in range(2) for qr in range(NS)]

            def new_pts():
                return [work.tile([128, KC * QW], BF16, tag=f"pt{j}", bufs=3,
                                  name=f"pt_{it_n[0]}_{j}")
                        for j in range(2)]
            it_n = [0]

            with tc.tile_pool(name="pin", bufs=1) as pin:
                xt_all = pin.tile([128, 4, N], F16, name="xt_all")
                wkv_all = pin.tile([128, 4, 512], F16, name="wkv_all")
                xt_sb = [xt_all[:, c, :] for c in range(4)]
                wkv_sb = [wkv_all[:, c, :] for c in range(4)]
                qsl = [slice(QW * qr, QW * (qr + 1)) for qr in range(NS)]
                # DMA need-order: kproj(0,0) wants wkv K-cols + all of x
                # qr0; split those across the two HWDGE queues first.
                # PE warm-up: ~8 dummy matmuls on (garbage) kt2 so the HAM
                # clock gate reaches 8/8 (~3.4us of activity) before the
                # real projections arrive -- otherwise the whole startup
                # runs at the cold 1.2GHz half-rate. Output is never read.
                warm = mps.tile([128, GRPW * QW], F32, tag="sc", bufs=2,
                                name="warmup")
                for _ in range(8):
                    nc.tensor.matmul(warm[:, :QW], kt2[0][:, 0:128],
                                     kt2[0][:, 0:QW], start=True, stop=True)
                # DMA order: the first 8 issues get the 8 DMA completion
                # lanes; later ones wait for a recycled lane (so put the
                # late-needed xt3/bvb last). Completion receipt costs
                # ~2.5us on top of transfer, so the kproj(0,0) inputs are
                # split fine-grained across both HWDGE queues.
                nc.sync.dma_start(out=wkv_all[:, :, 0:256],
                                  in_=wkv[:, :, 0:256])
                nc.scalar.dma_start(out=xt_all[:, 0:1, qsl[0]],
                                    in_=xt4[0][:, 0:1, :])
                nc.sync.dma_start(out=xt_all[:, 2:3, qsl[0]],
                                  in_=xt4[0][:, 2:3, :])
                nc.scalar.dma_start(out=xt_all[:, 1:2, qsl[0]],
                                    in_=xt4[0][:, 1:2, :])
                nc.sync.dma_start(out=xt_all[:, 3:4, qsl[0]],
                                  in_=xt4[0][:, 3:4, :])
                for p in range(2):
                    nc.scalar.dma_start(out=bk_sb[p], in_=bk2[p])
                nc.sync.dma_start(out=xt_all[:, :, qsl[1]], in_=xt4[1])
                nc.scalar.dma_start(out=wkv_all[:, :, 256:512],
                                    in_=wkv[:, :, 256:512])
                nc.scalar.dma_start(out=xt_all[:, :, qsl[2]], in_=xt4[2])
                nc.sync.dma_start(out=xt_all[:, :, qsl[3]], in_=xt4[3])
                nc.gpsimd.dma_start(out=bvb_sb, in_=bvb[:])
                # relaxed-deadline bd zero halves (p=1: first read ~iter 4)
                nc.gpsimd.memset(bd[0][1][64:128, :], 0.0)
                nc.gpsimd.memset(bd[1][1][0:64, :], 0.0)

                # ---- iteration 0 (p0, qr0): ALL projections ride along
                # on the free "av" psum banks, so the score/exp stream
                # through "sc" is never displaced. Group split [2,2,...]
                # so the first TWO groups need only kproj(0,0) (the later
                # xt q-ranges are still in flight at ~15us). ----
                it_n[0] = 0
                GROUPS0 = [(0, 2), (2, 2), (4, 3), (7, 3), (10, 3), (13, 3)]
                pts_prev = new_pts()
                kproj(0, 0)
                scores_group(0, 0, GROUPS0[0], pts_prev)
                kproj(0, 1)
                scores_group(0, 0, GROUPS0[1], pts_prev)
                for t in (0, 1, 2):
                    vproj(t)
                scores_group(0, 0, GROUPS0[2], pts_prev)
                kproj(0, 2)
                for t in (3, 4):
                    vproj(t)
                scores_group(0, 0, GROUPS0[3], pts_prev)
                kproj(0, 3)
                for t in (5, 6):
                    vproj(t)
                scores_group(0, 0, GROUPS0[4], pts_prev)
                kproj(1, 0)
                kproj(1, 1)
                for t in (7, 8):
                    vproj(t)
                scores_group(0, 0, GROUPS0[5], pts_prev)
                kproj(1, 2)
                kproj(1, 3)

                # ---- iterations 1..6: scores stream; prev iteration's
                # attV tail + epilogue slot in after g1 (by then ACT holds
                # a deep queue, so the PE detour can't starve it) ----
                st_A = None        # attV accumulator needing its tail
                pending_E = None   # (avs, rr) ready for epilogue
                for it in range(1, 7):
                    p, qr = ITERS[it]
                    it_n[0] = it
                    pts_cur = new_pts()
                    scores_group(p, qr, GROUPS[0], pts_cur)
                    if it == 1:
                        for t in (9, 10, 11, 12):
                            vproj(t)
                    scores_group(p, qr, GROUPS[1], pts_cur)
                    if it == 1:
                        for t in (13, 14, 15):
                            vproj(t)
                    if st_A is not None:
                        attv_chunks(st_A, AV_TAIL)
                        stc = attv_copies(st_A)
                        if pending_E is not None:
                            epilogue(pending_E)
                            pending_E = None
                        pending_E = attv_recip(stc)
                    st_B = attv_begin(*ITERS[it - 1], pts_prev)
                    for slot in range(4):
                        scores_group(p, qr, GROUPS[2 + slot], pts_cur)
                        attv_chunks(st_B, AV_PLAN[slot])
                    st_A = st_B
                    pts_prev = pts_cur

                # ---- iteration 7 (compressed ending) ----
                p, qr = ITERS[7]
                it_n[0] = 7
                pts_cur = new_pts()
                scores_group(p, qr, GROUPS[0], pts_cur)
                scores_group(p, qr, GROUPS[1], pts_cur)
                attv_chunks(st_A, AV_TAIL)              # attV(5) tail
                stc = attv_copies(st_A)
                epilogue(pending_E)                     # out(4)
                pending_E = attv_recip(stc)             # (5)
                st_B = attv_begin(*ITERS[6], pts_prev)  # attV(6), fast
                scores_group(p, qr, GROUPS[2], pts_cur)
                attv_chunks(st_B, (0, 1, 2, 3, 4, 5))
                scores_group(p, qr, GROUPS[3], pts_cur)
                attv_chunks(st_B, (6, 7, 8, 9, 10, 11))
                scores_group(p, qr, GROUPS[4], pts_cur)
                attv_chunks(st_B, AV_TAIL)
                stc6 = attv_copies(st_B)
                epilogue(pending_E)                     # out(5)
                pending6 = attv_recip(stc6)             # (6)
                scores_group(p, qr, GROUPS[5], pts_cur)
                st7 = attv_begin(p, qr, pts_cur)        # attV(7), lag-1
                attv_chunks(st7, tuple(range(0, 14)))
                epilogue(pending6, last=True)           # out(6) rides the
                attv_chunks(st7, (14, 15))              # last-exp window
                stc7 = attv_copies(st7)
                epilogue(attv_recip(stc7), last=True)   # out(7)

    nc.finalize()
    return nc


_program = None


def kernel(x, Wk, bk, Wv, bv):
    global _program, _last_results
    x = np.asarray(x, dtype=np.float32)
    Wk = np.asarray(Wk, dtype=np.float32)
    bk = np.asarray(bk, dtype=np.float32)
    Wv = np.asarray(Wv, dtype=np.float32)
    bv = np.asarray(bv, dtype=np.float32)

    if _program is None:
        _program = build_program()

    sq = np.float32(1.0 / np.sqrt(E))
    in_maps = []
    for c in range(NCORES):
        b, hg = c // 2, c % 2
        cols = slice(hg * HPC * D, (hg + 1) * HPC * D)
        wkvm = np.concatenate(
            [Wk[cols, :].T, Wv[cols, :].T * sq], axis=1)          # [E, 512]
        # [E, 512] -> [c, p, j] -> [p, c, j] fp16 (4KB contiguous/partition)
        wkv_h = np.ascontiguousarray(
            wkvm.reshape(4, 128, 512).transpose(1, 0, 2)).astype(np.float16)
        # x[b].T: [E, N] -> [c, p, qr, i] -> [qr, p, c, i] fp16
        xt_h = np.ascontiguousarray(
            x[b].T.reshape(4, 128, NS, QW).transpose(2, 1, 0, 3)
        ).astype(np.float16)
        in_maps.append({
            "xt4": xt_h,
            "wkv": wkv_h,
            "bk2": np.ascontiguousarray(bk[cols].reshape(2, 128, 1)),
            "bvb": np.ascontiguousarray(
                np.broadcast_to(bv[cols] * sq, (128, HPC * D))),
        })

    import os
    trace = bool(int(os.environ.get("KERNEL_PROFILE", "0")))
    res = run_bass_kernel_spmd(_program, in_maps, list(range(NCORES)),
                               trace=trace)
    _last_results = res

    out = np.empty((B, N, E), dtype=np.float32)
    for c in range(NCORES):
        b, hg = c // 2, c % 2
        ot = res.results[c]["out_t"]                              # [4, 64, N]
        for hl in range(HPC):
            out[b, :, hg * HPC * D + hl * D:(hg * HPC * D) + (hl + 1) * D] = \
                ot[hl].T.astype(np.float32)
    return out


# revision 35
# speedup vs baseline: 1.0018x; 1.0018x over previous
"""Trainium2 Bass kernel for BudgetAttentionTwo (v5).

Module: keys = x@Wk.T+bk, values = x@Wv.T+bv (split into 8 heads of 64),
S = K K^T per (b, h), out = (softmax(S)/sqrt(E)) @ V, merged back to [B,N,E].

Sharding: 8 cores, each core owns one batch b = core//2 and four heads
hg*4..hg*4+3 (hg = core%2). No cross-device comms.

History (per-core HW time):
  v3 197.9us -> v4 187.8 (fp16 inputs/outputs, contiguous DMA layouts)
  -> v8 165.3 -> v11 161.4. The steady state is ACT-bound: exp of 16.8M
  scores = 96 ACTIVATEs x (FD+296 cyc)/1.2GHz = 132.9us with <1us of
  stalls; the rest is startup (~16us: preamble 6, DMA transfer+receipt
  ~7, first kproj/score chain ~3) and tail (~12: last attV + DVE
  normalize chain + DMA receipt + ~5us teardown).

Key mechanics (learned the hard way, via ntff traces):
  - attV tail chunks (12..15) of iteration k-1 run after iteration k's
    g0/g1 score groups (pts pool bufs=3) so PE detours never starve ACT;
    groups [2,3,3,3,3,2] make the boundary group cheap.
  - ALL projections live in iteration 0 on the "av" psum banks (attV has
    no accumulator yet), so kproj/vproj never displace a score buffer.
  - reciprocal_approx_fast (custom DVE, base partition 0 ONLY -- offset
    64 returns garbage on HW) + bf16 cast; the broadcast matmul must be
    bf16 (fp32 matmuls lower to a HI/LO pair at ~3x cost).
  - K=64 sub-array score matmuls (tile_position row 64) corrupt on HW
    even though CoreSim accepts them -- hence the bd zero-half tensors.
  - 8 warm-up matmuls on garbage bring the PE HAM clock gate to 2.4GHz
    before the first real projection.
  - DMA: fp16, partition-major 4KB lines; first 8 dma_starts get the 8
    completion lanes; ~2.5us receipt latency per transfer.
Numerics: fp16 x/W projections measured 5.04e-3 rel err (tolerance 2e-2),
bit-identical to the numpy simulation of the dtype chain.

P is bf16, V (with trailing ones column) bf16, K fp16. exp(S - 88) is
exact for softmax (max logit ~131); rowsums via the ones column; output
stays transposed [64 d, N] fp16 per head; host transposes and upcasts.
"""
import numpy as np

import concourse.bacc as bacc
import concourse.mybir as mybir
import concourse.tile as tile
from concourse.bass_utils import run_bass_kernel_spmd

F32 = mybir.dt.float32
BF16 = mybir.dt.bfloat16
F16 = mybir.dt.float16
EXP = mybir.ActivationFunctionType.Exp

B, N, E, H = 4, 2048, 512, 8
D = E // H            # 64
NCORES = 8
HPC = 4               # heads per core
CSHIFT = 88.0         # exp(S - CSHIFT)
QW = 512              # q-range width
NS = N // QW          # 4 q-ranges
KC = N // 128         # 16 k-chunks
GRPW = 3              # max k-chunks per psum tile / exp call
GROUPS = [(0, 2), (2, 3), (5, 3), (8, 3), (11, 3), (14, 2)]
# attV chunks of the accumulator begun last iteration, emitted after score
# groups g1..g4; the (12..15) tail runs after the NEXT iteration's g0.
AV_PLAN = [(0, 1, 2), (3, 4, 5), (6, 7, 8), (9, 10, 11)]
AV_TAIL = (12, 13, 14, 15)

_last_results = None  # stashed BassKernelResults for test.py introspection


def _register_const(nc, val):
    """Extra pre-TileContext f32 [128,1] constant (dep-free, like Bass's
    built-in consts) so activation(bias=val) needs no semaphore wait."""
    t = nc.alloc_sbuf_tensor(f"const-float32-{val}", [128, 1], F32)
    nc.gpsimd.memset(t.ap(), val)
    nc.const_aps.aps[(F32, float(val))] = t.ap()
    # no all_engine_barrier: the memset retires ~6us into the gpsimd
    # preamble, ~9us before the first ACTIVATE that reads it; the explicit
    # barrier cost 1.4us of startup


def build_program():
    nc = bacc.Bacc()
    _register_const(nc, -CSHIFT)

    xt4 = nc.dram_tensor("xt4", [NS, 128, 4, QW], F16, kind="ExternalInput")
    wkv = nc.dram_tensor("wkv", [128, 4, 512], F16, kind="ExternalInput")
    bk2 = nc.dram_tensor("bk2", [2, 128, 1], F32, kind="ExternalInput")
    bvb = nc.dram_tensor("bvb", [128, 2 * 128], F32, kind="ExternalInput")
    out_t = nc.dram_tensor("out_t", [HPC, D, N], F16, kind="ExternalOutput")

    with nc.allow_low_precision(reason="fp16/bf16 datapath is intentional"), \
         tile.TileContext(nc) as tc:
        with (
            tc.tile_pool(name="persist", bufs=1) as per,
            tc.tile_pool(name="work", bufs=2) as work,
            tc.tile_pool(name="mps", bufs=1, space="PSUM") as mps,
        ):
            # ---- persistent SBUF ----
            kt2 = [per.tile([128, N], F16, name=f"kt2_{p}") for p in range(2)]
            bd = [[per.tile([128, N], F16, name=f"bd_{j}_{p}")
                   for p in range(2)] for j in range(2)]
            vs = [per.tile([128, HPC * (D + 1)], BF16, name=f"vs_{t}")
                  for t in range(KC)]
            bvb_sb = per.tile([128, HPC * D], F32)
            bk_sb = [per.tile([128, 1], F32, name=f"bk_{p}") for p in range(2)]
            # ones rows at partitions 0 and 32 for the two broadcast
            # matmuls (sub-128 partition bases beyond 32 misbehave on HW);
            # bf16 so the broadcast matmul stays a single cheap instruction
            # (fp32 matmuls lower to a HI/LO pair at ~3x the cost)
            ones33 = per.tile([33, D], BF16)

            nc.gpsimd.memset(ones33[:], 1.0)
            # bd zero halves: only the p=0 pair is needed early (first
            # scores ~12us); p=1 isn't read until iteration 4 (~75us), so
            # those go on gpsimd AFTER its DMA issues (emitted below).
            # Keeping DVE to 2 memsets frees it for the first kproj add.
            nc.vector.memset(bd[0][0][64:128, :], 0.0)
            nc.vector.memset(bd[1][0][0:64, :], 0.0)

            def kproj(p, qr):
                # psum from the "av" tag: free during iteration 0 (no attV
                # yet), so projections never displace a score buffer
                acc = mps.tile([128, QW], F32, tag="av", bufs=2,
                               name=f"kacc_{p}_{qr}")
                for c in range(4):
                    nc.tensor.matmul(
                        acc[:],
                        wkv_sb[c][:, 128 * p:128 * (p + 1)],
                        xt_sb[c][:, QW * qr:QW * (qr + 1)],
                        start=(c == 0), stop=(c == 3),
                    )
                qs = slice(QW * qr, QW * (qr + 1))
                nc.vector.tensor_scalar_add(kt2[p][:, qs], acc[:],
                                            bk_sb[p][:])
                # bd halves: fast DVE f16 copies from kt2 (4x perf mode),
                # off the psum (which the single add above already drained)
                nc.vector.tensor_copy(bd[0][p][0:64, qs], kt2[p][0:64, qs])
                nc.vector.tensor_copy(bd[1][p][64:128, qs],
                                      kt2[p][64:128, qs])

            def vproj(t):
                acc = mps.tile([128, QW], F32, tag="av", bufs=2,
                               name=f"vacc_{t}")
                for c in range(4):
                    nc.tensor.matmul(
                        acc[:, :HPC * D],
                        xt_sb[c][:, 128 * t:128 * (t + 1)],
                        wkv_sb[c][:, 256:512],
                        start=(c == 0), stop=(c == 3),
                    )
                vst = vs[t].rearrange("p (h y) -> p h y", h=HPC)
                nc.gpsimd.memset(vst[:, :, D], 1.0)
                nc.vector.tensor_tensor(
                    out=vst[:, :, 0:D],
                    in0=acc[:, :HPC * D].rearrange("p (h d) -> p h d", h=HPC),
                    in1=bvb_sb.rearrange("p (h d) -> p h d", h=HPC),
                    op=mybir.AluOpType.add,
                )

            def scores_group(p, qr, gw, pts):
                """Score matmuls + exp for k-chunks g..g+w-1 of (p, qr).
                Full K=128 contraction with the bd zero-half trick (K=64
                sub-array matmuls are broken on HW; CoreSim disagrees)."""
                g, w = gw
                sc = [mps.tile([128, GRPW * QW], F32, tag="sc", bufs=2,
                               name=f"sc_{p}_{qr}_{g}_{j}")
                      for j in range(2)]
                for i in range(w):
                    kc = g + i
                    for j in range(2):
                        nc.tensor.matmul(
                            sc[j][:, QW * i:QW * (i + 1)],
                            kt2[p][:, 128 * kc:128 * (kc + 1)],
                            bd[j][p][:, QW * qr:QW * (qr + 1)],
                            start=True, stop=True,
                        )
                for j in range(2):
                    nc.scalar.activation(
                        pts[j][:, QW * g:QW * (g + w)],
                        sc[j][:, :QW * w],
                        EXP, bias=-CSHIFT, scale=1.0,
                    )

            def attv_begin(p, cc, pts):
                return {
                    "p": p, "cc": cc, "pts": pts, "n": [0, 0],
                    "av": [mps.tile([128, QW], F32, tag="av", bufs=2,
                                    name=f"av_{p}_{cc}_{j}")[0:D + 1, :]
                           for j in range(2)],
                }

            def attv_chunks(st, kcs):
                p = st["p"]
                for kc in kcs:
                    vsl = vs[kc].rearrange("p (h y) -> p h y", h=HPC)
                    for j in range(2):
                        nc.tensor.matmul(
                            st["av"][j][:], vsl[:, 2 * p + j, :],
                            st["pts"][j][:, QW * kc:QW * (kc + 1)],
                            start=(st["n"][j] == 0),
                            stop=(st["n"][j] == KC - 1),
                        )
                        st["n"][j] += 1

            def attv_copies(st):
                p, cc = st["p"], st["cc"]
                assert st["n"] == [KC, KC]
                avs = []
                rb = work.tile([33, QW], F32, tag="rb", bufs=2,
                               name=f"rb_{p}_{cc}")
                for j in range(2):
                    av_sb = work.tile([D + 1, QW], F32, tag="avsb", bufs=4,
                                      name=f"avsb_{p}_{cc}_{j}")
                    nc.vector.tensor_copy(av_sb[:], st["av"][j][:])
                    nc.vector.tensor_copy(rb[32 * j:32 * j + 1, :],
                                          av_sb[D:D + 1, :])
                    avs.append(av_sb)
                return (p, cc, avs, rb)

            def attv_recip(stc):
                """Reciprocal of both rowsum rows (lanes 0 and 32; lanes
                1..31 hold junk whose reciprocal is never read), then a
                bf16 cast so the broadcast matmul stays single-instruction."""
                p, cc, avs, rb = stc
                rr = work.tile([33, QW], F32, tag="rr", bufs=2,
                               name=f"rr_{p}_{cc}")
                nc.vector.reciprocal_approx_fast(rr[:], rb[:])
                rrb = work.tile([33, QW], BF16, tag="rrb", bufs=2,
                                name=f"rrb_{p}_{cc}")
                nc.vector.tensor_copy(rrb[:], rr[:])
                return (p, cc, avs, rrb)

            def epilogue(state, last=False):
                p, cc, avs, rr = state
                q0 = QW * cc
                for j in range(2):
                    hl = 2 * p + j
                    if last:
                        # scores are done; the freed "sc" banks host the
                        # broadcast so it needn't wait for the av rotation
                        bc = mps.tile([128, GRPW * QW], F32, tag="sc",
                                      bufs=2,
                                      name=f"bc_{p}_{cc}_{j}")[0:D, 0:QW]
                    else:
                        bc = mps.tile([128, QW], F32, tag="av", bufs=2,
                                      name=f"bc_{p}_{cc}_{j}")[0:D, :]
                    nc.tensor.matmul(bc[:], ones33[32 * j:32 * j + 1, :],
                                     rr[32 * j:32 * j + 1, :],
                                     start=True, stop=True)
                    fin = work.tile([D, QW], F16, tag="fin", bufs=2,
                                    name=f"fin_{p}_{cc}_{j}")
                    nc.vector.tensor_tensor(
                        out=fin[:], in0=avs[j][0:D, :], in1=bc[:],
                        op=mybir.AluOpType.mult)
                    eng = nc.scalar if last else nc.sync
                    eng.dma_start(
                        out=out_t[hl, :, q0:q0 + QW], in_=fin[:])

            ITERS = [(p, qr) for p in range(2) for qr in range(NS)]

            def new_pts():
                return [work.tile([128, KC * QW], BF16, tag=f"pt{j}", bufs=3,
                                  name=f"pt_{it_n[0]}_{j}")
                        for j in range(2)]
            it_n = [0]

            with tc.tile_pool(name="pin", bufs=1) as pin:
                xt_all = pin.tile([128, 4, N], F16, name="xt_all")
                wkv_all = pin.tile([128, 4, 512], F16, name="wkv_all")
                xt_sb = [xt_all[:, c, :] for c in range(4)]
                wkv_sb = [wkv_all[:, c, :] for c in range(4)]
                qsl = [slice(QW * qr, QW * (qr + 1)) for qr in range(NS)]
                # DMA need-order: kproj(0,0) wants wkv K-cols + all of x
                # qr0; split those across the two HWDGE queues first.
                # PE warm-up: ~8 dummy matmuls on (garbage) kt2 so the HAM
                # clock gate reaches 8/8 (~3.4us of activity) before the
                # real projections arrive -- otherwise the whole startup
                # runs at the cold 1.2GHz half-rate. Output is never read.
                warm = mps.tile([128, GRPW * QW], F32, tag="sc", bufs=2,
                                name="warmup")
                for _ in range(8):
                    nc.tensor.matmul(warm[:, :QW], kt2[0][:, 0:128],
                                     kt2[0][:, 0:QW], start=True, stop=True)
                # DMA order: the first 8 issues get the 8 DMA completion
                # lanes; later ones wait for a recycled lane (so put the
                # late-needed xt3/bvb last). Completion receipt costs
                # ~2.5us on top of transfer, so the kproj(0,0) inputs are
                # split fine-grained across both HWDGE queues.
                nc.sync.dma_start(out=wkv_all[:, :, 0:256],
                                  in_=wkv[:, :, 0:256])
                nc.scalar.dma_start(out=xt_all[:, 0:1, qsl[0]],
                                    in_=xt4[0][:, 0:1, :])
                nc.sync.dma_start(out=xt_all[:, 2:3, qsl[0]],
                                  in_=xt4[0][:, 2:3, :])
                nc.scalar.dma_start(out=xt_all[:, 1:2, qsl[0]],
                                    in_=xt4[0][:, 1:2, :])
                nc.sync.dma_start(out=xt_all[:, 3:4, qsl[0]],
                                  in_=xt4[0][:, 3:4, :])
                for p in range(2):
                    nc.scalar.dma_start(out=bk_sb[p], in_=bk2[p])
                nc.sync.dma_start(out=xt_all[:, :, qsl[1]], in_=xt4[1])
                nc.scalar.dma_start(out=wkv_all[:, :, 256:512],
                                    in_=wkv[:, :, 256:512])
                nc.scalar.dma_start(out=xt_all[:, :, qsl[2]], in_=xt4[2])
                nc.sync.dma_start(out=xt_all[:, :, qsl[3]], in_=xt4[3])
                nc.gpsimd.dma_start(out=bvb_sb, in_=bvb[:])
                # relaxed-deadline bd zero halves (p=1: first read ~iter 4)
                nc.gpsimd.memset(bd[0][1][64:128, :], 0.0)
                nc.gpsimd.memset(bd[1][1][0:64, :], 0.0)

                # ---- iteration 0 (p0, qr0): ALL projections ride along
                # on the free "av" psum banks, so the score/exp stream
                # through "sc" is never displaced ----
                it_n[0] = 0
                GROUPS0 = GROUPS
                pts_prev = new_pts()
                kproj(0, 0)
                scores_group(0, 0, GROUPS0[0], pts_prev)
                kproj(0, 1)
                scores_group(0, 0, GROUPS0[1], pts_prev)
                for t in (0, 1, 2):
                    vproj(t)
                scores_group(0, 0, GROUPS0[2], pts_prev)
                kproj(0, 2)
                for t in (3, 4):
                    vproj(t)
                scores_group(0, 0, GROUPS0[3], pts_prev)
                kproj(0, 3)
                for t in (5, 6):
                    vproj(t)
                scores_group(0, 0, GROUPS0[4], pts_prev)
                kproj(1, 0)
                kproj(1, 1)
                for t in (7, 8):
                    vproj(t)
                scores_group(0, 0, GROUPS0[5], pts_prev)
                kproj(1, 2)
                kproj(1, 3)

                # ---- iterations 1..6: scores stream; prev iteration's
                # attV tail + epilogue slot in after g1 (by then ACT holds
                # a deep queue, so the PE detour can't starve it) ----
                st_A = None        # attV accumulator needing its tail
                pending_E = None   # (avs, rr) ready for epilogue
                for it in range(1, 7):
                    p, qr = ITERS[it]
                    it_n[0] = it
                    pts_cur = new_pts()
                    scores_group(p, qr, GROUPS[0], pts_cur)
                    if it == 1:
                        for t in (9, 10, 11, 12):
                            vproj(t)
                    scores_group(p, qr, GROUPS[1], pts_cur)
                    if it == 1:
                        for t in (13, 14, 15):
                            vproj(t)
                    if st_A is not None:
                        attv_chunks(st_A, AV_TAIL)
                        stc = attv_copies(st_A)
                        if pending_E is not None:
                            epilogue(pending_E)
                            pending_E = None
                        pending_E = attv_recip(stc)
                    st_B = attv_begin(*ITERS[it - 1], pts_prev)
                    for slot in range(4):
                        scores_group(p, qr, GROUPS[2 + slot], pts_cur)
                        attv_chunks(st_B, AV_PLAN[slot])
                    st_A = st_B
                    pts_prev = pts_cur

                # ---- iteration 7 (compressed ending) ----
                p, qr = ITERS[7]
                it_n[0] = 7
                pts_cur = new_pts()
                scores_group(p, qr, GROUPS[0], pts_cur)
                scores_group(p, qr, GROUPS[1], pts_cur)
                attv_chunks(st_A, AV_TAIL)              # attV(5) tail
                stc = attv_copies(st_A)
                epilogue(pending_E)                     # out(4)
                pending_E = attv_recip(stc)             # (5)
                st_B = attv_begin(*ITERS[6], pts_prev)  # attV(6), fast
                scores_group(p, qr, GROUPS[2], pts_cur)
                attv_chunks(st_B, (0, 1, 2, 3, 4, 5))
                scores_group(p, qr, GROUPS[3], pts_cur)
                attv_chunks(st_B, (6, 7, 8, 9, 10, 11))
                scores_group(p, qr, GROUPS[4], pts_cur)
                attv_chunks(st_B, AV_TAIL)
                stc6 = attv_copies(st_B)
                epilogue(pending_E)                     # out(5)
                pending6 = attv_recip(stc6)             # (6)
                scores_group(p, qr, GROUPS[5], pts_cur)
                st7 = attv_begin(p, qr, pts_cur)        # attV(7), lag-1
                attv_chunks(st7, tuple(range(0, 14)))
                epilogue(pending6, last=True)           # out(6) rides the
                attv_chunks(st7, (14, 15))              # last-exp window
                stc7 = attv_copies(st7)
                epilogue(attv_recip(stc7), last=True)   # out(7)

    nc.finalize()
    return nc


_program = None


def kernel(x, Wk, bk, Wv, bv):
    global _program, _last_results
    x = np.asarray(x, dtype=np.float32)
    Wk = np.asarray(Wk, dtype=np.float32)
    bk = np.asarray(bk, dtype=np.float32)
    Wv = np.asarray(Wv, dtype=np.float32)
    bv = np.asarray(bv, dtype=np.float32)

    if _program is None:
        _program = build_program()

    sq = np.float32(1.0 / np.sqrt(E))
    in_maps = []
    for c in range(NCORES):
        b, hg = c // 2, c % 2
        cols = slice(hg * HPC * D, (hg + 1) * HPC * D)
        wkvm = np.concatenate(
            [Wk[cols, :].T, Wv[cols, :].T * sq], axis=1)          # [E, 512]
        # [E, 512] -> [c, p, j] -> [p, c, j] fp16 (4KB contiguous/partition)
        wkv_h = np.ascontiguousarray(
            wkvm.reshape(4, 128, 512).transpose(1, 0, 2)).astype(np.float16)
        # x[b].T: [E, N] -> [c, p, qr, i] -> [qr, p, c, i] fp16
        xt_h = np.ascontiguousarray(
            x[b].T.reshape(4, 128, NS, QW).transpose(2, 1, 0, 3)
        ).astype(np.float16)
        in_maps.append({
            "xt4": xt_h,
            "wkv": wkv_h,
            "bk2": np.ascontiguousarray(bk[cols].reshape(2, 128, 1)),
            "bvb": np.ascontiguousarray(
                np.broadcast_to(bv[cols] * sq, (128, HPC * D))),
        })

    import os
    trace = bool(int(os.environ.get("KERNEL_PROFILE", "0")))
    res = run_bass_kernel_spmd(_program, in_maps, list(range(NCORES)),
                               trace=trace)
    _last_results = res

    out = np.empty((B, N, E), dtype=np.float32)
    for c in range(NCORES):
        b, hg = c // 2, c % 2
        ot = res.results[c]["out_t"]                              # [4, 64, N]
        for hl in range(HPC):
            out[b, :, hg * HPC * D + hl * D:(hg * HPC * D) + (hl + 1) * D] = \
                ot[hl].T.astype(np.float32)
    return out
